# revision 10
# baseline (speedup 1.0000x reference)
"""Trainium2 Bass kernel for nn_BeliefPlausibilityFocused.

reference():
    cardinal_fod = inputs.shape[-1] - 1 = 3; n_sets = 8
    bel[..., j] = 1.0 if (j & focal) == focal else 0.0
    pl[...,  j] = 1.0 if (j & focal) >  0    else 0.0
Both outputs are per-pixel broadcast constants of shape
inputs.shape[:-1] + (8,) = [8, 384, 1248, 8]; the input VALUES are unused.

Strategy (pure data-parallel over batch, per sharding hint):
  - 8 cores, one batch element each. Per-core output: bel/pl each
    [384, 1248, 8] f32 = 15.3 MB -> 30.7 MB HBM writes per core.
  - Host computes the two 8-float masks from `focal` and passes one tiny
    [128, 16] pattern tensor. Device replicates it along the free dim with
    log-doubling DVE copies into two SBUF tiles, then issues large HWDGE
    DMA stores to fill the outputs. Memory-roofline ~= 30.7MB / ~358 GB/s.
"""

import numpy as np

import concourse.bacc as bacc
import concourse.mybir as mybir
import concourse.tile as tile
from concourse.bass_utils import run_bass_kernel_spmd

# Problem shapes (hardcoded per contract: kernel.py must be self-contained).
B, H, W, C = 8, 384, 1248, 4
NSETS = 1 << (C - 1)          # 8
N_CORES = 8
P = 128                        # SBUF partitions

PIX = H * W                    # 479232 pixels per batch element
PER_OUT = PIX * NSETS          # 3,833,856 f32 per output per core
PER_PART = PER_OUT // P        # 29,952 f32 per partition
SRC_F = 3744                   # source tile width; 14976 B per repeat chunk
REP = PER_PART // SRC_F        # 32 stride-0 repeats per store

assert PER_OUT % P == 0 and PER_PART % NSETS == 0 and SRC_F % NSETS == 0
assert SRC_F * REP == PER_PART

_NC_CACHE = {}
LAST_RESULTS = None  # BassKernelResults of the most recent run (for test.py)


def _build_nc(bel_mask, pl_mask):
    """One SPMD program: memset-build the two 8-float mask patterns in SBUF,
    then store each output with one big stride-0-source DMA. The mask values
    (derived from `focal` on the host) are baked into the program."""
    nc = bacc.Bacc(None, target_bir_lowering=False)

    bel = nc.dram_tensor("bel", [P, PER_PART], mybir.dt.float32,
                         kind="ExternalOutput")
    pl = nc.dram_tensor("pl", [P, PER_PART], mybir.dt.float32,
                        kind="ExternalOutput")

    with tile.TileContext(nc) as tc:
        with tc.tile_pool(name="sbuf", bufs=1) as pool:
            belt = pool.tile([P, SRC_F], mybir.dt.float32, tag="belt")
            plt = pool.tile([P, SRC_F], mybir.dt.float32, tag="plt")

            # Build each 8-periodic pattern: bulk memset of the majority
            # value, then strided memsets for the minority channels.
            # belt on DVE, plt on GpSimd so they fill in parallel.
            for t, mask, eng in ((belt, bel_mask, nc.vector),
                                 (plt, pl_mask, nc.gpsimd)):
                ones = [int(c) for c in np.nonzero(mask)[0]]
                zeros = [c for c in range(NSETS) if c not in ones]
                maj, minority = (1.0, zeros) if len(ones) >= len(zeros) \
                    else (0.0, ones)
                eng.memset(t[:], maj)
                t3 = t[:].rearrange("p (r c) -> p r c", c=NSETS)
                for c in minority:
                    # integer index -> squeezed 2D strided AP (3D count-1
                    # APs hard-fault the engines)
                    eng.memset(t3[:, :, c], 1.0 - maj)

            # One store per output; the source AP repeats the small tile via
            # a stride-0 dim. bel on the SP ring, pl on the ACT ring.
            for t, out, eng in ((belt, bel, nc.sync), (plt, pl, nc.scalar)):
                o3 = out[:].rearrange("p (r f) -> p r f", r=REP)
                src = t[:].unsqueeze(1).broadcast_to([P, REP, SRC_F])
                eng.dma_start(out=o3, in_=src)

    nc.finalize()
    return nc


def _get_nc(bel_mask, pl_mask):
    key = (tuple(bel_mask), tuple(pl_mask))
    if key not in _NC_CACHE:
        _NC_CACHE[key] = _build_nc(bel_mask, pl_mask)
    return _NC_CACHE[key]


def kernel(inputs, focal):
    global LAST_RESULTS
    inputs = np.asarray(inputs)
    focal_i = int(np.asarray(focal))
    assert inputs.shape == (B, H, W, C), inputs.shape

    # Host-side mask computation (cheap: 8 elements).
    j = np.arange(NSETS, dtype=np.int64)
    contain = j & focal_i
    bel_mask = (contain == focal_i).astype(np.float32)
    pl_mask = (contain > 0).astype(np.float32)

    nc = _get_nc(bel_mask, pl_mask)
    in_maps = [{} for _ in range(N_CORES)]
    res = run_bass_kernel_spmd(nc, in_maps, list(range(N_CORES)))
    LAST_RESULTS = res

    out_dtype = inputs.dtype
    bel_full = np.empty((B, H, W, NSETS), dtype=out_dtype)
    pl_full = np.empty((B, H, W, NSETS), dtype=out_dtype)
    for b in range(N_CORES):
        bel_full[b] = res.results[b]["bel"].reshape(H, W, NSETS)
        pl_full[b] = res.results[b]["pl"].reshape(H, W, NSETS)
    return (bel_full, pl_full)


# revision 11
# speedup vs baseline: 1.2146x; 1.2146x over previous
"""Trainium2 Bass kernel for nn_BeliefPlausibilityFocused.

reference():
    cardinal_fod = inputs.shape[-1] - 1 = 3; n_sets = 8
    bel[..., j] = 1.0 if (j & focal) == focal else 0.0
    pl[...,  j] = 1.0 if (j & focal) >  0    else 0.0
Both outputs are per-pixel broadcast constants of shape
inputs.shape[:-1] + (8,) = [8, 384, 1248, 8]; the input VALUES are unused.

Strategy (pure data-parallel over batch, per sharding hint):
  - 8 cores, one batch element each. Per-core output: bel/pl each
    [384, 1248, 8] f32 = 15.3 MB -> 30.7 MB HBM writes per core.
  - Host computes the two 8-float masks from `focal` and passes one tiny
    [128, 16] pattern tensor. Device replicates it along the free dim with
    log-doubling DVE copies into two SBUF tiles, then issues large HWDGE
    DMA stores to fill the outputs. Memory-roofline ~= 30.7MB / ~358 GB/s.
"""

import numpy as np

import concourse.bacc as bacc
import concourse.mybir as mybir
import concourse.tile as tile
from concourse.bass_utils import run_bass_kernel_spmd

# Problem shapes (hardcoded per contract: kernel.py must be self-contained).
B, H, W, C = 8, 384, 1248, 4
NSETS = 1 << (C - 1)          # 8
N_CORES = 8
P = 128                        # SBUF partitions

PIX = H * W                    # 479232 pixels per batch element
PER_OUT = PIX * NSETS          # 3,833,856 f32 per output per core
PER_PART = PER_OUT // P        # 29,952 f32 per partition
SRC_F = 1872                   # source tile width; 7488 B per repeat chunk
REP = PER_PART // SRC_F        # 32 stride-0 repeats per store

assert PER_OUT % P == 0 and PER_PART % NSETS == 0 and SRC_F % NSETS == 0
assert SRC_F * REP == PER_PART

_NC_CACHE = {}
LAST_RESULTS = None  # BassKernelResults of the most recent run (for test.py)


def _build_nc(bel_mask, pl_mask):
    """One SPMD program: memset-build the two 8-float mask patterns in SBUF,
    then store each output with one big stride-0-source DMA. The mask values
    (derived from `focal` on the host) are baked into the program."""
    nc = bacc.Bacc(None, target_bir_lowering=False)

    bel = nc.dram_tensor("bel", [P, PER_PART], mybir.dt.float32,
                         kind="ExternalOutput")
    pl = nc.dram_tensor("pl", [P, PER_PART], mybir.dt.float32,
                        kind="ExternalOutput")

    with tile.TileContext(nc) as tc:
        with tc.tile_pool(name="sbuf", bufs=1) as pool:
            belt = pool.tile([P, SRC_F], mybir.dt.float32, tag="belt")
            plt = pool.tile([P, SRC_F], mybir.dt.float32, tag="plt")

            # Build each 8-periodic pattern: bulk memset of the majority
            # value, then strided memsets for the minority channels.
            # belt on DVE, plt on GpSimd so they fill in parallel.
            for t, mask, eng in ((belt, bel_mask, nc.vector),
                                 (plt, pl_mask, nc.gpsimd)):
                ones = [int(c) for c in np.nonzero(mask)[0]]
                zeros = [c for c in range(NSETS) if c not in ones]
                maj, minority = (1.0, zeros) if len(ones) >= len(zeros) \
                    else (0.0, ones)
                eng.memset(t[:], maj)
                t3 = t[:].rearrange("p (r c) -> p r c", c=NSETS)
                for c in minority:
                    # integer index -> squeezed 2D strided AP (3D count-1
                    # APs hard-fault the engines)
                    eng.memset(t3[:, :, c], 1.0 - maj)

            # One store per output; the source AP repeats the small tile via
            # a stride-0 dim. bel on the SP ring, pl on the ACT ring.
            for t, out, eng in ((belt, bel, nc.sync), (plt, pl, nc.scalar)):
                o3 = out[:].rearrange("p (r f) -> p r f", r=REP)
                src = t[:].unsqueeze(1).broadcast_to([P, REP, SRC_F])
                eng.dma_start(out=o3, in_=src)

    nc.finalize()
    return nc


def _get_nc(bel_mask, pl_mask):
    key = (tuple(bel_mask), tuple(pl_mask))
    if key not in _NC_CACHE:
        _NC_CACHE[key] = _build_nc(bel_mask, pl_mask)
    return _NC_CACHE[key]


def kernel(inputs, focal):
    global LAST_RESULTS
    inputs = np.asarray(inputs)
    focal_i = int(np.asarray(focal))
    assert inputs.shape == (B, H, W, C), inputs.shape

    # Host-side mask computation (cheap: 8 elements).
    j = np.arange(NSETS, dtype=np.int64)
    contain = j & focal_i
    bel_mask = (contain == focal_i).astype(np.float32)
    pl_mask = (contain > 0).astype(np.float32)

    nc = _get_nc(bel_mask, pl_mask)
    in_maps = [{} for _ in range(N_CORES)]
    res = run_bass_kernel_spmd(nc, in_maps, list(range(N_CORES)))
    LAST_RESULTS = res

    out_dtype = inputs.dtype
    bel_full = np.empty((B, H, W, NSETS), dtype=out_dtype)
    pl_full = np.empty((B, H, W, NSETS), dtype=out_dtype)
    for b in range(N_CORES):
        bel_full[b] = res.results[b]["bel"].reshape(H, W, NSETS)
        pl_full[b] = res.results[b]["pl"].reshape(H, W, NSETS)
    return (bel_full, pl_full)
